# revision 28
# baseline (speedup 1.0000x reference)
"""Trainium2 Bass kernel for nn_MultiModalFusionModelWithAblation.

Strategy: pure data-parallel over 8 NeuronCores (B=16384 -> 2048 rows/core).

v2 layout/engine plan (vs v1):
  - All activations/weights pre-cast to bf16 and pre-transposed ON HOST into
    the exact SBUF layouts the matmuls consume:
      featT  [NT/2, 128, 44, 256]  feature-major, per-256-row supertile,
                                   contiguous 22.5 KB per partition per DMA
      wq     [128, 37032]          one blob: wp|aw1|aw2|gv|gs|wo|pc views
      lgst   [NT, 60, 128]         aux logits/scores pre-transposed
    This kills the per-tile SWDGE cast-DMA descriptor storm (15k+ tiny
    descriptors per engine) and 7 of the 12 per-tile DMA transposes.
  - Bulk loads ride SWDGE (gpsimd); the two HWDGE rings (sync / scalar)
    carry only the 5 remaining SBUF transposes, split across both rings.
  - Scalar engine uses ONLY {Relu, Copy, Ln, Exp, Square} = one activation
    table set (natural_log_exp_and_others): rsqrt = exp(-0.5*ln(v+eps)),
    sigmoid = 1/(1+exp(-x)) with the reciprocal on DVE.  v1 thrashed
    Sqrt/Exp/Sigmoid sets at ~2.7us per reload, ~120us/core.
  - Host-side algebra (exact, weight-space only) unchanged from v1:
    gat_W folded into score/value projections, biases via K=1 ones-matmuls
    (skipped when zero), guide rows pre-normalized.
"""
import sys
import os

sys.path.insert(0, "/opt/trn_rl_repo")

import numpy as np
import ml_dtypes
import orjson
from contextlib import ExitStack

import concourse.bass as bass
import concourse.tile as tile
from concourse import mybir

# ----------------------------------------------------------------------------
# walrus on this toolchain rejects >1 sync-wait per instruction; split excess
# waits onto NoOp carriers on the same engine queue (in-order => equivalent).
_FIXN = [0]


def _fix_bir_waits(d):
    for f in d.get("functions", []):
        for b in f.get("blocks", []):
            insts = b.get("instructions", [])
            if not any(
                len(((i.get("sync_info") or {}).get("on_wait") or [])) > 1
                for i in insts
            ):
                continue
            new = []
            for inst in insts:
                si = inst.get("sync_info")
                waits = (si or {}).get("on_wait") or []
                if len(waits) > 1:
                    for w in waits[:-1]:
                        _FIXN[0] += 1
                        new.append({
                            "engine": inst["engine"], "ins": [], "outs": [],
                            "name": f"wfix-{_FIXN[0]}", "opcode": "NoOp",
                            "debug": inst.get("debug", 0),
                            "sync_info": {"on_update": [], "on_wait": [w]},
                        })
                    si["on_wait"] = [waits[-1]]
                new.append(inst)
            b["instructions"] = new
    return d


if not getattr(bass.Bass, "_waitfix_installed", False):
    _orig_tjb = bass.Bass.to_json_bytes

    def _patched_tjb(self):
        return orjson.dumps(_fix_bir_waits(orjson.loads(_orig_tjb(self))))

    bass.Bass.to_json_bytes = _patched_tjb
    bass.Bass._waitfix_installed = True

# ----------------------------------------------------------------------------
H = 512
NH = 8
HD = 64
NMOD = 5
IN_DIMS = [2048, 1024, 1536, 512, 512]
MODS = ["body", "face", "scene", "audio", "text"]
B_FULL = 16384
NCORES = 8
B_CORE = B_FULL // NCORES          # 2048
NT = B_CORE // 128                 # 16 row tiles per core
NT2 = NT // 2                      # 8 supertiles (256 rows)
ALPHA = 0.2
EPS = 1e-5

NK = [d // 128 for d in IN_DIMS]   # [16, 8, 12, 4, 4]
NKS = sum(NK)                      # 44
K0 = [sum(NK[:m]) for m in range(NMOD)]

# wq blob element offsets (per partition)
OFF_WP = 0
OFF_A1 = OFF_WP + NKS * H                  # 22528
OFF_A2 = OFF_A1 + NMOD * 4 * 256           # 27648
OFF_GV = OFF_A2 + NMOD * 2 * H             # 32768
OFF_GS = OFF_GV + 4 * H                    # 34816
OFF_WO = OFF_GS + 4 * 18                   # 34888
OFF_PC = OFF_WO + 4 * H                    # 36936
WQ_F = OFF_PC + 4 * 24                     # 37032

DEBUG_DUMP = False

F32 = mybir.dt.float32
BF16 = mybir.dt.bfloat16
AF = mybir.ActivationFunctionType
AL = mybir.AluOpType


def _build_nc(flags):
    nc = bass.Bass("TRN2", target_bir_lowering=False, debug=False,
                   num_devices=NCORES)

    # ---- dram io ----
    featT_d = nc.dram_tensor("featT", [NT, 128, NKS, 128], BF16,
                             kind="ExternalInput")
    wq_d = nc.dram_tensor("wq", [128, WQ_F], BF16, kind="ExternalInput")
    elpl_d = nc.dram_tensor("elpl", [35, 2 * H], BF16, kind="ExternalInput")
    lgst_d = nc.dram_tensor("lgst", [NT, 60, 128], BF16, kind="ExternalInput")
    # optional bias rows (always declared; tiny)
    bp_d = nc.dram_tensor("bp", [NMOD, H], F32, kind="ExternalInput")
    ab1_d = nc.dram_tensor("ab1e", [NMOD, H // 2], F32, kind="ExternalInput")
    ab2_d = nc.dram_tensor("ab2e", [NMOD, H], F32, kind="ExternalInput")
    rc_d = nc.dram_tensor("rc", [2, H], F32, kind="ExternalInput")
    pcb_d = nc.dram_tensor("pcb", [1, 24], F32, kind="ExternalInput")
    ck_d = nc.dram_tensor("ck", [1, 16], F32, kind="ExternalInput")
    out_d = nc.dram_tensor("out", [B_CORE, 12], F32, kind="ExternalOutput")
    dbg = {}
    if DEBUG_DUMP:
        for name, shape, dt in [
            ("d_hln", [NT, 128, NMOD, H], BF16),
            ("d_xm", [NT, 128, NMOD, H], BF16),
            ("d_xss", [NT, 128, NMOD, 18], F32),
            ("d_xvt", [NT, 128, NMOD, H], BF16),
            ("d_W", [NT, 128, 16, 5], BF16),
            ("d_opair", [NT, 128, 2, H], BF16),
            ("d_rep", [NT, 128, 2, H], BF16),
            ("d_pred", [NT, 128, 24], F32),
            ("d_hT", [NT, 128, NMOD * 4, 128], BF16),
            ("d_z", [NT, 128, NMOD, 256], BF16),
            ("d_zT", [NT, 128, NMOD * 2, 128], BF16),
        ]:
            dbg[name] = nc.dram_tensor(name, shape, dt, kind="ExternalOutput")

    with tile.TileContext(nc) as tc, ExitStack() as ctx:
        wp_pool = ctx.enter_context(tc.tile_pool(name="weights", bufs=1))
        sb = ctx.enter_context(tc.tile_pool(name="work", bufs=1))
        ps = ctx.enter_context(tc.tile_pool(name="psum", bufs=1, space="PSUM"))

        # double-buffered tile loads (defined+issued before weights so the
        # first feat tile lands early)
        def load_tile(rt):
            fs = sb.tile([128, NKS, 128], BF16, tag="fstage", bufs=2)
            nc.gpsimd.dma_start(fs[:], featT_d.ap()[rt])
            return fs

        def load_pair(p):
            # lt/st in separate tiles so both sit at base partition 0
            # (matmul lhsT must share base_partition with rhs)
            lg = sb.tile([35, 2, 128], BF16, tag="lg2", bufs=6)
            nc.gpsimd.dma_start(
                lg[:], lgst_d.ap()[2 * p:2 * p + 2, 0:35].rearrange(
                    "t q r -> q t r"))
            sc = sb.tile([25, 2, 128], BF16, tag="sc2", bufs=6)
            nc.gpsimd.dma_start(
                sc[:], lgst_d.ap()[2 * p:2 * p + 2, 35:60].rearrange(
                    "t q r -> q t r"))
            return lg, sc

        tile_bufs = {0: load_tile(0)}
        pair_tiles = {0: load_pair(0)}

        # ---- one-time weight loads (SWDGE bulk; no casts needed) ----
        # split so tile 0's projection can start after ~3.5 MB, not 9.5 MB
        wq_t = wp_pool.tile([128, WQ_F], BF16, tag="wq")
        WPB = NK[0] * H
        nc.gpsimd.dma_start(wq_t[:, :WPB], wq_d.ap()[:, :WPB])
        nc.gpsimd.dma_start(wq_t[:, WPB:OFF_A1], wq_d.ap()[:, WPB:OFF_A1])
        nc.gpsimd.dma_start(wq_t[:, OFF_A1:], wq_d.ap()[:, OFF_A1:])
        elpl_t = wp_pool.tile([35, 2 * H], BF16, tag="elpl")
        nc.gpsimd.dma_start(elpl_t[:], elpl_d.ap()[:])

        wp_v = wq_t[:, OFF_WP:OFF_A1].rearrange("p (k n) -> p k n", n=H)
        a1_v = wq_t[:, OFF_A1:OFF_A2].rearrange(
            "p (m k n) -> p m k n", m=NMOD, k=4)
        a2_v = wq_t[:, OFF_A2:OFF_GV].rearrange(
            "p (m k n) -> p m k n", m=NMOD, k=2)
        gv_v = wq_t[:, OFF_GV:OFF_GS].rearrange("p (k n) -> p k n", k=4)
        gs_v = wq_t[:, OFF_GS:OFF_WO].rearrange("p (k n) -> p k n", k=4)
        wo_v = wq_t[:, OFF_WO:OFF_PC].rearrange("p (k n) -> p k n", k=4)
        pc_v = wq_t[:, OFF_PC:WQ_F].rearrange("p (k n) -> p k n", k=4)

        eps_t = wp_pool.tile([128, 1], F32, tag="eps")
        nc.vector.memset(eps_t[:], EPS)

        ones1 = None
        if any([flags["bp"], flags["ab1"], flags["ab2"], flags["rc"],
                flags["pcb"], flags["ck"]]):
            ones1 = wp_pool.tile([1, 128], BF16, tag="ones1")
            nc.vector.memset(ones1[:], 1.0)

        def _bias_row(dram_ap, n, tag):
            t = wp_pool.tile([1, n], BF16, tag=tag)
            nc.gpsimd.dma_start(t[:], dram_ap)
            return t

        bp_bf = _bias_row(bp_d.ap().rearrange("m n -> 1 (m n)"), NMOD * H,
                          "bp") if flags["bp"] else None
        ab1_bf = _bias_row(ab1_d.ap().rearrange("m n -> 1 (m n)"), NMOD * 256,
                           "ab1") if flags["ab1"] else None
        ab2_bf = _bias_row(ab2_d.ap().rearrange("m n -> 1 (m n)"), NMOD * H,
                           "ab2") if flags["ab2"] else None
        rc_bf = _bias_row(rc_d.ap().rearrange("q n -> 1 (q n)"), 2 * H,
                          "rc") if flags["rc"] else None
        pcb_bf = _bias_row(pcb_d.ap()[:], 24, "pcb") if flags["pcb"] else None
        ck_t = None
        if flags["ck"]:
            ck_row = _bias_row(ck_d.ap()[:], 16, "ckrow")
            ck_ps = ps.tile([128, 16], F32, tag="psCK")
            nc.tensor.matmul(ck_ps[:], lhsT=ones1[:], rhs=ck_row[:],
                             start=True, stop=True)
            ck_t = wp_pool.tile([128, 16], F32, tag="ckt")
            nc.vector.tensor_copy(out=ck_t[:], in_=ck_ps[:])

        # output accumulator for the single final store
        outt_all = wp_pool.tile([128, NT, 12], F32, tag="outt")


        def layer_norm(src_tiles, dst_tag, pfx, s1, s2, dst_bufs=2):
            # stats from ACT-accumulated sums: mean = s1/H,
            # var = s2/H - mean^2; rsqrt(v+eps) = exp(-0.5*ln(v+eps))
            # (stays in the exp/ln ACT table set)
            out = sb.tile([128, NMOD, H], BF16, tag=dst_tag, bufs=dst_bufs)
            mean5 = sb.tile([128, NMOD], F32, tag=pfx + "m", bufs=2)
            nc.vector.tensor_scalar_mul(mean5[:], s1[:], 1.0 / H)
            t2 = sb.tile([128, NMOD], F32, tag=pfx + "t", bufs=2)
            nc.vector.tensor_scalar_mul(t2[:], s2[:], 1.0 / H)
            msq = sb.tile([128, NMOD], F32, tag=pfx + "q", bufs=2)
            nc.vector.tensor_tensor(out=msq[:], in0=mean5[:], in1=mean5[:],
                                    op=AL.mult)
            var5 = sb.tile([128, NMOD], F32, tag=pfx + "v", bufs=2)
            nc.vector.tensor_tensor(out=var5[:], in0=t2[:], in1=msq[:],
                                    op=AL.subtract)
            lv = sb.tile([128, NMOD], F32, tag=pfx + "l", bufs=2)
            nc.scalar.activation(lv[:], var5[:], AF.Ln, bias=eps_t[:])
            rsa = sb.tile([128, NMOD], F32, tag=pfx + "r", bufs=2)
            nc.scalar.activation(rsa[:], lv[:], AF.Exp, scale=-0.5)
            for m in range(NMOD):
                nc.vector.tensor_scalar(out=out[:, m, :], in0=src_tiles[m][:],
                                        scalar1=mean5[:, m:m + 1],
                                        scalar2=rsa[:, m:m + 1],
                                        op0=AL.subtract, op1=AL.mult)
            return out

        # ---------------- per row-tile pipeline ----------------
        def emit_A(rt):
            if rt + 1 < NT:
                tile_bufs[rt + 1] = load_tile(rt + 1)
            if rt % 2 == 0 and rt // 2 + 1 < NT2:
                pair_tiles[rt // 2 + 1] = load_pair(rt // 2 + 1)
            fs = tile_bufs.pop(rt)
            lg, sc = pair_tiles[rt // 2]
            s1a = sb.tile([128, NMOD], F32, tag="s1a", bufs=2)
            h_sb = []
            for m in range(NMOD):
                h_ps = ps.tile([128, H], F32, tag="psA", bufs=2)
                if flags["bp"]:
                    nc.tensor.matmul(h_ps[:], lhsT=ones1[:],
                                     rhs=bp_bf[:, m * H:(m + 1) * H],
                                     start=True, stop=False)
                for k in range(NK[m]):
                    nc.tensor.matmul(h_ps[:], lhsT=fs[:, K0[m] + k, :],
                                     rhs=wp_v[:, K0[m] + k, :],
                                     start=(k == 0 and not flags["bp"]),
                                     stop=(k == NK[m] - 1))
                hs = sb.tile([128, H], BF16, tag="h_sb", bufs=5)
                nc.scalar.activation(hs[:], h_ps[:], AF.Relu,
                                     accum_out=s1a[:, m:m + 1])
                h_sb.append(hs)
            return dict(rt=rt, h_sb=h_sb, s1a=s1a, lt=lg[0:35, rt % 2, :],
                        st=sc[0:25, rt % 2, :])

        def emit_LN1(state):
            s2a = sb.tile([128, NMOD], F32, tag="s2a", bufs=2)
            for m in range(NMOD):
                sqh = sb.tile([128, H], BF16, tag="sqh", bufs=1)
                nc.scalar.activation(sqh[:], state["h_sb"][m][:], AF.Square,
                                     accum_out=s2a[:, m:m + 1])
            hln = layer_norm(state["h_sb"], "hln", "sa", state["s1a"], s2a)
            hT = sb.tile([128, NMOD * 4, 128], BF16, tag="hT", bufs=2)
            nc.sync.dma_start(hT[:], hln[:].rearrange("p m h -> p (m h)"),
                              transpose=True)
            state["hln"] = hln
            state["hT"] = hT
            if DEBUG_DUMP:
                nc.sync.dma_start(dbg["d_hln"].ap()[state["rt"]], hln[:])
                nc.sync.dma_start(dbg["d_hT"].ap()[state["rt"]], hT[:])
            return state

        def emit_C(state):
            hT, hln = state["hT"], state["hln"]
            z = sb.tile([128, NMOD, 256], BF16, tag="z", bufs=2)
            for m in range(NMOD):
                a1_ps = ps.tile([128, 256], F32, tag="psB", bufs=2)
                if flags["ab1"]:
                    nc.tensor.matmul(a1_ps[:], lhsT=ones1[:],
                                     rhs=ab1_bf[:, m * 256:(m + 1) * 256],
                                     start=True, stop=False)
                for k in range(4):
                    nc.tensor.matmul(a1_ps[:], lhsT=hT[:, m * 4 + k, :],
                                     rhs=a1_v[:, m, k, :],
                                     start=(k == 0 and not flags["ab1"]),
                                     stop=(k == 3))
                nc.scalar.activation(z[:, m, :], a1_ps[:], AF.Relu)
            zT = sb.tile([128, NMOD * 2, 128], BF16, tag="zT", bufs=1)
            nc.sync.dma_start(zT[:], z[:].rearrange("p m h -> p (m h)"),
                              transpose=True)
            if DEBUG_DUMP:
                nc.sync.dma_start(dbg["d_z"].ap()[state["rt"]], z[:])
                nc.sync.dma_start(dbg["d_zT"].ap()[state["rt"]], zT[:])
            us = []
            s1b = sb.tile([128, NMOD], F32, tag="s1b", bufs=2)
            for m in range(NMOD):
                a2_ps = ps.tile([128, H], F32, tag="psC", bufs=2)
                if flags["ab2"]:
                    nc.tensor.matmul(a2_ps[:], lhsT=ones1[:],
                                     rhs=ab2_bf[:, m * H:(m + 1) * H],
                                     start=True, stop=False)
                for k in range(2):
                    nc.tensor.matmul(a2_ps[:], lhsT=zT[:, m * 2 + k, :],
                                     rhs=a2_v[:, m, k, :],
                                     start=(k == 0 and not flags["ab2"]),
                                     stop=(k == 1))
                a2c = sb.tile([128, H], BF16, tag="a2c", bufs=2)
                nc.scalar.activation(a2c[:], a2_ps[:], AF.Copy,
                                     accum_out=s1b[:, m:m + 1])
                u = sb.tile([128, H], BF16, tag="u", bufs=5)
                nc.gpsimd.tensor_tensor(out=u[:], in0=a2c[:],
                                        in1=hln[:, m, :], op=AL.add)
                us.append(u)
            state["us"] = us
            state["s1b"] = s1b
            return state

        def emit_LN2(state):
            s2b = sb.tile([128, NMOD], F32, tag="s2b", bufs=2)
            for m in range(NMOD):
                sqh = sb.tile([128, H], BF16, tag="sqh", bufs=1)
                nc.scalar.activation(sqh[:], state["us"][m][:], AF.Square,
                                     accum_out=s2b[:, m:m + 1])
            xm = layer_norm(state["us"], "xm", "sb", state["s1b"], s2b,
                            dst_bufs=1)
            xT = sb.tile([128, NMOD * 4, 128], BF16, tag="xT", bufs=2)
            nc.sync.dma_start(xT[:], xm[:].rearrange("p m h -> p (m h)"),
                              transpose=True)
            state["xT"] = xT
            if DEBUG_DUMP:
                nc.sync.dma_start(dbg["d_xm"].ap()[state["rt"]], xm[:])
            return state

        def emit_E(state):
            xT = state["xT"]
            xss = sb.tile([128, NMOD, 18], BF16, tag="xss", bufs=2)
            xvt = sb.tile([128, NMOD, H], BF16, tag="xvt", bufs=3)
            xs_ps = ps.tile([128, NMOD, 18], F32, tag="psB", bufs=2)
            for m in range(NMOD):
                xv_ps = ps.tile([128, H], F32, tag="psC", bufs=2)
                for k in range(4):
                    nc.tensor.matmul(xv_ps[:], lhsT=xT[:, m * 4 + k, :],
                                     rhs=gv_v[:, k, :],
                                     start=(k == 0), stop=(k == 3))
                for k in range(4):
                    nc.tensor.matmul(xs_ps[:, m, :], lhsT=xT[:, m * 4 + k, :],
                                     rhs=gs_v[:, k, :],
                                     start=(k == 0), stop=(k == 3))
                nc.scalar.activation(xvt[:, m, :], xv_ps[:], AF.Copy)
            nc.vector.tensor_copy(out=xss[:], in_=xs_ps[:])
            state.update(xss=xss, xvt=xvt)
            if DEBUG_DUMP:
                nc.sync.dma_start(dbg["d_xss"].ap()[state["rt"]], xss[:])
                nc.sync.dma_start(dbg["d_xvt"].ap()[state["rt"]], xvt[:])
            return state

        def emit_attn(state):
            """GAT attention softmaxes + pooled-attention weights.

            attn is never materialized: S = (ex @ s16) * rden and
            W = (P * rden16 * rden-by-n) @ ex^T fold the normalization in.
            """
            xss = state["xss"]

            e = sb.tile([128, 5, 5], F32, tag="e", bufs=2)
            nc.gpsimd.tensor_tensor(
                out=e[:],
                in0=xss[:, :, 16:17].broadcast_to([128, 5, 5]),
                in1=xss[:, None, :, 17].broadcast_to([128, 5, 5]),
                op=AL.add)
            el = sb.tile([128, 25], F32, tag="el", bufs=2)
            nc.vector.scalar_tensor_tensor(
                out=el[:], in0=e[:].rearrange("p a b -> p (a b)"), scalar=ALPHA,
                in1=e[:].rearrange("p a b -> p (a b)"), op0=AL.mult, op1=AL.max)
            ex = sb.tile([128, 5, 5], F32, tag="ex", bufs=2)
            nc.scalar.activation(ex[:].rearrange("p a b -> p (a b)"), el[:],
                                 AF.Exp)
            den = sb.tile([128, 5], F32, tag="den", bufs=2)
            nc.vector.tensor_reduce(out=den[:], in_=ex[:],
                                    axis=mybir.AxisListType.X, op=AL.add)
            rden = sb.tile([128, 5], F32, tag="rden", bufs=2)
            nc.vector.reciprocal(rden[:], den[:])
            attn = sb.tile([128, 5, 5], BF16, tag="attn", bufs=2)
            nc.gpsimd.tensor_tensor(
                out=attn[:], in0=ex[:],
                in1=rden[:, :, None].broadcast_to([128, 5, 5]), op=AL.mult)

            # S[q16, n] = sum_j attn[n, j] * s16[j, q]
            tmp400 = sb.tile([128, 16, 5, 5], BF16, tag="tmp400", bufs=1)
            nc.gpsimd.tensor_tensor(
                out=tmp400[:],
                in0=xss[:, :, 0:16].rearrange("p j q -> p q j")[:, :, None, :]
                    .broadcast_to([128, 16, 5, 5]),
                in1=attn[:][:, None, :, :].broadcast_to([128, 16, 5, 5]),
                op=AL.mult)
            S = sb.tile([128, 16, 5], F32, tag="S", bufs=1)
            nc.vector.tensor_reduce(out=S[:], in_=tmp400[:],
                                    axis=mybir.AxisListType.X, op=AL.add)
            if flags["ck"]:
                nc.vector.tensor_tensor(
                    out=S[:], in0=S[:],
                    in1=ck_t[:][:, :, None].broadcast_to([128, 16, 5]),
                    op=AL.add)
            ES = sb.tile([128, 16, 5], F32, tag="ES", bufs=1)
            nc.scalar.activation(ES[:].rearrange("p a b -> p (a b)"),
                                 S[:].rearrange("p a b -> p (a b)"), AF.Exp)
            den16 = sb.tile([128, 16], F32, tag="den16", bufs=2)
            nc.vector.tensor_reduce(out=den16[:], in_=ES[:],
                                    axis=mybir.AxisListType.X, op=AL.add)
            rden16 = sb.tile([128, 16], F32, tag="rden16", bufs=2)
            nc.vector.reciprocal(rden16[:], den16[:])
            P = sb.tile([128, 16, 5], BF16, tag="P", bufs=2)
            nc.gpsimd.tensor_tensor(
                out=P[:], in0=ES[:],
                in1=rden16[:, :, None].broadcast_to([128, 16, 5]), op=AL.mult)

            # W[q16, j] = sum_n P[q, n] * attn[n, j]
            tmp2 = sb.tile([128, 16, 5, 5], BF16, tag="tmp400", bufs=1)
            nc.gpsimd.tensor_tensor(
                out=tmp2[:],
                in0=P[:][:, :, None, :].broadcast_to([128, 16, 5, 5]),
                in1=attn[:].rearrange("p n j -> p j n")[:, None, :, :]
                    .broadcast_to([128, 16, 5, 5]),
                op=AL.mult)
            W = sb.tile([128, 16, 5], BF16, tag="W", bufs=2)
            with nc.allow_low_precision("5-term pooled-attn sums"):
                nc.vector.tensor_reduce(out=W[:], in_=tmp2[:],
                                        axis=mybir.AxisListType.X, op=AL.add)
            state["W"] = W
            if DEBUG_DUMP:
                nc.sync.dma_start(dbg["d_W"].ap()[state["rt"]], W[:])
            return state

        def emit_P1(state):
            """Pooled values per query + per-q transposes."""
            xvt, W = state["xvt"], state["W"]
            o_pair = sb.tile([128, 2, H], BF16, tag="o_pair", bufs=2)
            oTs = []
            for q in range(2):
                tmp_o = sb.tile([128, NH, HD, 5], BF16, tag="tmp_o", bufs=2)
                nc.vector.tensor_tensor(
                    out=tmp_o[:],
                    in0=xvt[:].rearrange("p j (h d) -> p h d j", h=NH),
                    in1=W[:, q * 8:(q + 1) * 8, None, :]
                        .broadcast_to([128, NH, HD, 5]),
                    op=AL.mult)
                with nc.allow_low_precision("5-term pooled-attn sums"):
                    nc.vector.tensor_reduce(
                        out=o_pair[:, q, :].rearrange("p (h d) -> p h d", h=NH),
                        in_=tmp_o[:], axis=mybir.AxisListType.X, op=AL.add)
                oT = sb.tile([128, 4, 128], BF16, tag=f"oT{q}", bufs=2)
                nc.sync.dma_start(oT[:], o_pair[:, q, :], transpose=True)
                oTs.append(oT)
            state["oTs"] = oTs
            return state

        def emit_P2(state):
            """Out-proj + aux-logit terms + norms (ACT-only chain)."""
            oTs, lt, st = state["oTs"], state["lt"], state["st"]
            rep_pair = sb.tile([128, 2, H], BF16, tag="rep_pair", bufs=1)
            n2 = sb.tile([128, 2], F32, tag="n2", bufs=2)
            rTs = []
            for q in range(2):
                repr_ps = ps.tile([128, H], F32, tag="psD", bufs=2)
                if flags["rc"]:
                    nc.tensor.matmul(repr_ps[:], lhsT=ones1[:],
                                     rhs=rc_bf[:, q * H:(q + 1) * H],
                                     start=True, stop=False)
                for k in range(4):
                    nc.tensor.matmul(repr_ps[:], lhsT=oTs[q][:, k, :],
                                     rhs=wo_v[:, k, :],
                                     start=(k == 0 and not flags["rc"]),
                                     stop=False)
                if q == 0:
                    nc.tensor.matmul(repr_ps[:], lhsT=lt,
                                     rhs=elpl_t[0:35, 0:H],
                                     start=False, stop=True)
                else:
                    nc.tensor.matmul(repr_ps[:], lhsT=st,
                                     rhs=elpl_t[0:25, H:2 * H],
                                     start=False, stop=True)
                nc.scalar.activation(rep_pair[:, q, :], repr_ps[:], AF.Copy)
                sqd = sb.tile([128, H], BF16, tag="tmp_o", bufs=2)
                nc.scalar.activation(sqd[:], repr_ps[:], AF.Square,
                                     accum_out=n2[:, q:q + 1])
                rT = sb.tile([128, 4, 128], BF16, tag=f"rT{q}", bufs=2)
                nc.sync.dma_start(rT[:], rep_pair[:, q, :], transpose=True)
                rTs.append(rT)
            lnn = sb.tile([128, 2], F32, tag="lnn", bufs=2)
            nc.scalar.activation(lnn[:], n2[:], AF.Ln, bias=eps_t[:])
            rn = sb.tile([128, 2], F32, tag="rn", bufs=2)
            nc.scalar.activation(rn[:], lnn[:], AF.Exp, scale=-0.5)
            state["rTs"] = rTs
            state["rn"] = rn
            return state

        def emit_P3(state):
            """Heads + output assembly."""
            rTs, rn, rt = state["rTs"], state["rn"], state["rt"]
            pred_ps = ps.tile([128, 24], F32, tag="psB", bufs=2)
            if flags["pcb"]:
                nc.tensor.matmul(pred_ps[:], lhsT=ones1[:], rhs=pcb_bf[:],
                                 start=True, stop=False)
            for q in range(2):
                cols = slice(0, 14) if q == 0 else slice(14, 24)
                for k in range(4):
                    nc.tensor.matmul(pred_ps[:, cols], lhsT=rTs[q][:, k, :],
                                     rhs=pc_v[:, k, cols],
                                     start=(k == 0 and not flags["pcb"]),
                                     stop=(k == 3))
            pred = sb.tile([128, 24], F32, tag="pred", bufs=2)
            nc.vector.tensor_copy(out=pred[:], in_=pred_ps[:])
            # emo_final = (pred_head + cos * rn0) * 0.5; pc already holds
            # emo_head_w * 0.5 and gn_emo^T * 0.5
            nc.vector.scalar_tensor_tensor(
                out=outt_all[:, rt, 0:7], in0=pred[:, 7:14],
                scalar=rn[:, 0:1], in1=pred[:, 0:7],
                op0=AL.mult, op1=AL.add)
            # pkl_final = (sigmoid(pred_head) + sigmoid(cos * rn1)) * 0.5
            # sigmoid(x) = 1 / (1 + exp(-x)): exp on ACT, reciprocal on DVE
            # pc cols 19:24 hold -gn_pkl^T (host-negated) so the sigmoid
            # argument needs no negation here
            ecp = sb.tile([128, 2, 5], F32, tag="ecp", bufs=2)
            nc.scalar.activation(ecp[:, 0, :], pred[:, 19:24], AF.Exp,
                                 scale=rn[:, 1:2])
            nc.scalar.activation(ecp[:, 1, :], pred[:, 14:19], AF.Exp,
                                 scale=-1.0)
            dcp = sb.tile([128, 2, 5], F32, tag="dcp", bufs=2)
            nc.vector.tensor_scalar_add(dcp[:], ecp[:], 1.0)
            rcp = sb.tile([128, 2, 5], F32, tag="rcp", bufs=2)
            nc.vector.reciprocal(rcp[:].rearrange("p a b -> p (a b)"),
                                 dcp[:].rearrange("p a b -> p (a b)"))
            sum5 = sb.tile([128, 5], F32, tag="sum5", bufs=2)
            nc.vector.tensor_tensor(out=sum5[:], in0=rcp[:, 0, :],
                                    in1=rcp[:, 1, :], op=AL.add)
            nc.vector.tensor_scalar_mul(outt_all[:, rt, 7:12], sum5[:], 0.5)

        stages = [emit_A, emit_LN1, emit_C, emit_LN2, emit_E, emit_attn,
                  emit_P1, emit_P2, emit_P3]
        nstg = len(stages)
        states = {}
        for tick in range(NT + nstg - 1):
            for s_idx in reversed(range(nstg)):
                i = tick - s_idx
                if 0 <= i < NT:
                    if s_idx == 0:
                        states[i] = emit_A(i)
                    else:
                        states[i] = stages[s_idx](states[i])
            if tick - nstg + 1 >= 0:
                states.pop(tick - nstg + 1, None)

        # single final store of all outputs
        nc.sync.dma_start(out_d.ap().rearrange("(t r) c -> r t c", r=128),
                          outt_all[:])

    return nc


_CACHE = {}


def _host_prep(inputs):
    """Exact weight-space folding (identical math to v1)."""
    f32 = np.float32
    gat_W = inputs["gat_W"].astype(f32)
    gat_a = inputs["gat_a"].astype(f32)
    mha_in_w = inputs["mha_in_w"].astype(f32)
    mha_in_b = inputs["mha_in_b"].astype(f32)
    Wq, Wk, Wv = np.split(mha_in_w, 3, axis=1)
    bq, bk, bv = np.split(mha_in_b, 3)

    def score_mat(query):
        qv = (query.astype(f32) @ Wq + bq).reshape(NH, HD)
        A = np.stack([Wk[:, h * HD:(h + 1) * HD] @ qv[h] for h in range(NH)], 1)
        cK = np.array([bk[h * HD:(h + 1) * HD] @ qv[h] for h in range(NH)], f32)
        return A / np.sqrt(HD), cK / np.sqrt(HD)

    A_emo, ck_emo = score_mat(inputs["emo_query"])
    A_pkl, ck_pkl = score_mat(inputs["pkl_query"])
    gs = gat_W @ np.concatenate(
        [A_emo, A_pkl, gat_a[:H, None], gat_a[H:, None]], 1)
    gv = gat_W @ Wv
    ck = np.concatenate([ck_emo, ck_pkl]).astype(f32)

    ln1_g = inputs["ln1_g"].astype(f32)
    ln1_b = inputs["ln1_b"].astype(f32)
    ln2_g = inputs["ln2_g"].astype(f32)
    ln2_b = inputs["ln2_b"].astype(f32)
    ln1_trivial = np.allclose(ln1_g, 1.0) and np.allclose(ln1_b, 0.0)
    ln2_trivial = np.allclose(ln2_g, 1.0) and np.allclose(ln2_b, 0.0)
    if not (ln1_trivial and ln2_trivial):
        raise NotImplementedError("non-trivial LN affine not supported")

    aw1 = np.stack([np.diag(ln1_g[m]) @ inputs["aW1"][m].astype(f32)
                    for m in range(NMOD)])
    ab1e = inputs["ab1"].astype(f32) + np.einsum(
        "mk,mkn->mn", ln1_b, inputs["aW1"].astype(f32))
    aw2 = inputs["aW2"].astype(f32)
    ab2e = inputs["ab2"].astype(f32)

    mha_out_w = inputs["mha_out_w"].astype(f32)
    mha_out_b = inputs["mha_out_b"].astype(f32)
    rc = np.stack([
        mha_out_b + bv @ mha_out_w + inputs["elp_b"].astype(f32),
        mha_out_b + bv @ mha_out_w + inputs["plp_b"].astype(f32)])

    def norm_rows(g):
        g = g.astype(f32)
        n = np.maximum(np.linalg.norm(g, axis=-1, keepdims=True), 1e-8)
        return g / n

    gn_emo = norm_rows(inputs["guide_emo"])
    gn_pkl = norm_rows(inputs["guide_pkl"])
    pc = np.concatenate([
        inputs["emo_head_w"].astype(f32) * 0.5, gn_emo.T * 0.5,
        inputs["pkl_head_w"].astype(f32), -gn_pkl.T], 1)
    pcb = np.concatenate([
        inputs["emo_head_b"].astype(f32) * 0.5, np.zeros(7, f32),
        inputs["pkl_head_b"].astype(f32), np.zeros(5, f32)])

    elp5 = np.tile(inputs["elp_w"].astype(f32) / NMOD, (NMOD, 1))
    plp5 = np.tile(inputs["plp_w"].astype(f32) / NMOD, (NMOD, 1))

    bf16 = ml_dtypes.bfloat16

    def kmajor(w, nkc, n):
        # [nkc*128, n] -> [128, nkc, n]
        return np.ascontiguousarray(
            w.reshape(nkc, 128, n).transpose(1, 0, 2))

    wq_parts = []
    for m, mod in enumerate(MODS):
        wq_parts.append(kmajor(inputs[f"Wp_{mod}"].astype(f32), NK[m], H))
    wq_parts.append(np.ascontiguousarray(
        aw1.reshape(NMOD, 4, 128, 256).transpose(2, 0, 1, 3)))
    wq_parts.append(np.ascontiguousarray(
        aw2.reshape(NMOD, 2, 128, H).transpose(2, 0, 1, 3)))
    wq_parts.append(kmajor(gv, 4, H))
    wq_parts.append(kmajor(np.ascontiguousarray(gs), 4, 18))
    wq_parts.append(kmajor(mha_out_w, 4, H))
    wq_parts.append(kmajor(np.ascontiguousarray(pc), 4, 24))
    wq_blob = np.concatenate(
        [p.reshape(128, -1) for p in wq_parts], axis=1).astype(bf16)
    assert wq_blob.shape == (128, WQ_F)

    elpl = np.zeros((35, 2 * H), np.float32)
    elpl[:, 0:H] = elp5
    elpl[0:25, H:2 * H] = plp5
    elpl = elpl.astype(bf16)

    host = dict(
        wq=wq_blob, elpl=elpl,
        bp=np.ascontiguousarray(inputs["bp"], f32),
        ab1e=np.ascontiguousarray(ab1e, f32),
        ab2e=np.ascontiguousarray(ab2e, f32),
        rc=np.ascontiguousarray(rc, f32),
        pcb=np.ascontiguousarray(pcb[None, :], f32),
        ck=np.ascontiguousarray(ck[None, :], f32),
    )
    flags = dict(
        bp=not np.allclose(host["bp"], 0.0),
        ab1=not np.allclose(host["ab1e"], 0.0),
        ab2=not np.allclose(host["ab2e"], 0.0),
        rc=not np.allclose(host["rc"], 0.0),
        pcb=not np.allclose(host["pcb"], 0.0),
        ck=not np.allclose(host["ck"], 0.0),
    )
    return host, flags


def _prep_data(inputs):
    """Batch-data layout prep (pure layout/dtype, no arithmetic):
    featT [NT2g, 128, 44, 256] and lgst [NTg, 60, 128], both bf16, global."""
    bf16 = ml_dtypes.bfloat16
    ntg = B_FULL // 128
    parts = []
    for m, mod in enumerate(MODS):
        f = np.asarray(inputs[f"feat_{mod}"], np.float32)
        # [B, ind] -> [ntg, 128(p), NK, 128(r)]
        parts.append(f.reshape(ntg, 128, NK[m], 128).transpose(0, 3, 2, 1))
    featT = np.concatenate(parts, axis=2).astype(bf16)

    lg = np.asarray(inputs["emo_logits_all"], np.float32)   # [5, B, 7]
    sc = np.asarray(inputs["per_scores_all"], np.float32)   # [5, B, 5]
    lgT = lg.transpose(0, 2, 1).reshape(35, ntg, 128)
    scT = sc.transpose(0, 2, 1).reshape(25, ntg, 128)
    lgst = np.concatenate([lgT, scT], axis=0).transpose(1, 0, 2).astype(bf16)
    return np.ascontiguousarray(featT), np.ascontiguousarray(lgst)


def _run(inputs, **spmd_kwargs):
    from concourse.bass_utils import run_bass_kernel_spmd

    host, flags = _host_prep(inputs)
    featT, lgst = _prep_data(inputs)
    key = (tuple(sorted(flags.items())), DEBUG_DUMP)
    if key not in _CACHE:
        _CACHE[key] = _build_nc(flags)
    nc = _CACHE[key]

    in_maps = []
    for c in range(NCORES):
        im = dict(host)
        im["featT"] = np.ascontiguousarray(featT[c * NT:(c + 1) * NT])
        im["lgst"] = np.ascontiguousarray(lgst[c * NT:(c + 1) * NT])
        in_maps.append(im)

    res = run_bass_kernel_spmd(nc, in_maps, list(range(NCORES)), **spmd_kwargs)
    out = np.concatenate([res.results[c]["out"] for c in range(NCORES)], 0)
    return out, res


def kernel(**inputs):
    return _run(inputs)[0]
